# revision 21
# baseline (speedup 1.0000x reference)
"""Trainium2 Bass kernel for Llama GQA attention (no mask), 8-way tensor
parallel over KV heads.

Problem shapes (hardcoded):
  x  (2, 2048, 4096) f32
  wq (4096, 4096), wk (1024, 4096), wv (1024, 4096), wo (4096, 4096) f32
  NUM_HEADS=32, NUM_KV_HEADS=8, HEAD_DIM=128, GQA group g=4

Sharding: core c owns KV head c (4 Q heads). x replicated (pre-transposed
to xT on host), wq/wk/wv sharded on output dim (pre-transposed host-side),
wo sharded on input dim. Each core computes a partial (4096, 4096) output
(its heads' contribution through wo); host sums the 8 partials in fp32.

All tensors are bf16 (PSUM accumulation fp32): same PE rate as fp32r
(1 col/cycle) but half the DMA/SBUF traffic, which removes the phase-1
x-feed stalls the fp32 version had.

Structure:
  phase 1: q/k/v projections. Weights DMA'd per-k-tile on the gpsimd
    queue, wo prefetched right after; x tiles on the sync queue.
    vT -> v via PE transposes. PSUM evacuations split ACT/DVE.
  phase 2 (fused attention + output projection, software-pipelined):
    per (batch, tq-chunk, head): scores transposed ST = kT_tile.T @ qT
    into [128,1024] PSUM (2 k-tiles per matmul pair), batched exp ->
    p (bf16). Softmax denominator = DVE pairwise-tree sum of the 16 p
    tiles + ONE ones-matmul per head (instead of 16 PE den matmuls).
    S-pairs are emitted one step ahead of PV-pairs so the PE never
    waits on the ACT exp; den/rec/normalize of head m flush during
    head m+1; output-projection quarters of the previous (b,chunk)
    interleave after each head so DVE/ACT load stays smooth.
"""

import sys
from contextlib import ExitStack

import numpy as np
from ml_dtypes import bfloat16

sys.path.insert(0, "/opt/trn_rl_repo")

import concourse.bass as bass  # noqa: E402
import concourse.tile as tile  # noqa: E402
from concourse import bacc, mybir  # noqa: E402
from concourse.bass_utils import run_bass_kernel_spmd  # noqa: E402
from concourse.masks import make_identity  # noqa: E402

NCORES = 8
B, S, H = 2, 2048, 4096
T = B * S                      # 4096 flattened tokens
D = 128                        # head dim
G = 4                          # q heads per core (GQA group)
HK = 32                        # h k-tiles (4096 / 128)
TT = T // 128                  # 32 token tiles
NJ = T // 512                  # 8 token chunks of 512
SJ = S // 512                  # 4 tq chunks per batch
SI = S // 128                  # 16 tk tiles per batch
NG = SI // 2                   # 8 k-tile pairs per batch
SCALE = float(1.0 / np.sqrt(D))

F32 = mybir.dt.float32
BF16 = mybir.dt.bfloat16
COPY = mybir.ActivationFunctionType.Copy
EXP = mybir.ActivationFunctionType.Exp


def build_nc():
    nc = bacc.Bacc("TRN2", target_bir_lowering=False, debug=False,
                   enable_asserts=True, num_devices=NCORES)
    xt = nc.declare_dram_parameter("xt", [H, T], BF16, isOutput=False)
    wqt = nc.declare_dram_parameter("wqt", [H, G * D], BF16, isOutput=False)
    wkt = nc.declare_dram_parameter("wkt", [H, D], BF16, isOutput=False)
    wvt = nc.declare_dram_parameter("wvt", [H, D], BF16, isOutput=False)
    wot = nc.declare_dram_parameter("wot", [G * D, H], BF16, isOutput=False)
    ones = nc.declare_dram_parameter("ones", [128, 128], BF16, isOutput=False)
    out = nc.declare_dram_parameter("out", [T, H], BF16, isOutput=True)

    xt_r = xt.ap().rearrange("(k p) t -> p k t", p=128)     # [128, 32, T]
    wqt_r = wqt.ap().rearrange("(k p) m -> p k m", p=128)   # [128, 32, 512]
    wkt_r = wkt.ap().rearrange("(k p) m -> p k m", p=128)   # [128, 32, 128]
    wvt_r = wvt.ap().rearrange("(k p) m -> p k m", p=128)   # [128, 32, 128]
    wot_r = wot.ap().rearrange("(k p) n -> p k n", p=128)   # [128, 4, T]
    out_r = out.ap()

    with tile.TileContext(nc) as tc:
        with ExitStack() as ctx:
            persist = ctx.enter_context(tc.tile_pool(name="persist", bufs=1))
            q_sb = persist.tile([128, G, T], BF16)       # qT per head, 4MB
            k_sb = persist.tile([128, T], BF16)          # kT, 1MB
            v_sb = persist.tile([128, TT, D], BF16)      # v natural, 1MB
            wo_sb = persist.tile([128, G, T], BF16)      # woT resident, 4MB
            ones_sb = persist.tile([128, 128], BF16)
            nc.scalar.dma_start(out=ones_sb, in_=ones.ap())

            # ---------------- phase 1: projections ----------------
            with ExitStack() as c1:
                wpool = c1.enter_context(tc.tile_pool(name="wpool", bufs=1))
                xpool = c1.enter_context(tc.tile_pool(name="xpool", bufs=12))
                vstg = c1.enter_context(tc.tile_pool(name="vstg", bufs=2))
                ps1 = c1.enter_context(tc.tile_pool(name="ps1", bufs=1, space="PSUM"))
                pstr = c1.enter_context(tc.tile_pool(name="pstr", bufs=2, space="PSUM"))

                # one tile PER k-chunk: dependency tracking is
                # tile-granular, so a single big tile would make the first
                # matmul wait for ALL 32 chunk DMAs
                wq_t = [wpool.tile([128, G * D], BF16, name=f"wq{k}")
                        for k in range(HK)]
                wk_t = [wpool.tile([128, D], BF16, name=f"wk{k}")
                        for k in range(HK)]
                wv_t = [wpool.tile([128, D], BF16, name=f"wv{k}")
                        for k in range(HK)]
                ident = wpool.tile([128, 128], BF16)
                dummy = wpool.tile([1, 1], BF16)
                # all three tensors of chunk k go to the SAME queue,
                # alternating by k, so complete chunks arrive in the order
                # the j=0 matmuls consume them
                for k in range(HK):
                    q = nc.gpsimd if k % 2 == 0 else nc.scalar
                    q.dma_start(out=wq_t[k], in_=wqt_r[:, k, :])
                    q.dma_start(out=wk_t[k], in_=wkt_r[:, k, :])
                    q.dma_start(out=wv_t[k], in_=wvt_r[:, k, :])
                make_identity(nc, ident)

                def v_transpose(pj, pv_st):
                    # one-j-delayed so PE never waits on the DVE staging copy
                    vt_ps = pstr.tile([128, 4, 128], BF16)
                    for tt in range(4):
                        nc.tensor.transpose(
                            vt_ps[:, tt, :], pv_st[:, tt * 128:(tt + 1) * 128],
                            ident)
                    nc.scalar.activation(
                        out=v_sb[:, 4 * pj:4 * pj + 4, :], in_=vt_ps, func=COPY)

                prev_v = None
                for j in range(NJ):
                    tsl = slice(j * 512, (j + 1) * 512)
                    q_ps = [ps1.tile([128, 512], F32, name=f"q_ps{m}")
                            for m in range(G)]
                    k_ps = ps1.tile([128, 512], F32)
                    v_ps = ps1.tile([128, 512], F32)
                    for k in range(HK):
                        x_t = xpool.tile([128, 512], BF16)
                        nc.sync.dma_start(out=x_t, in_=xt_r[:, k, tsl])
                        st = k == 0
                        sp = k == HK - 1
                        for m in range(G):
                            nc.tensor.matmul(
                                q_ps[m], wq_t[k][:, m * D:(m + 1) * D], x_t,
                                start=st, stop=sp)
                        nc.tensor.matmul(k_ps, wk_t[k], x_t, start=st, stop=sp)
                        nc.tensor.matmul(v_ps, wv_t[k], x_t, start=st, stop=sp)
                        if k == 2 and prev_v is not None:
                            v_transpose(*prev_v)
                        # prefetch wo for phase 2, gated on a j==3 x tile.
                        # The gpsimd queue posts DMAs in relaxed order, so a
                        # copy BEFORE the dma_start does not delay it; a
                        # writer-after-writer dependency on wo_sb itself does.
                        if j == 3 and k == 0:
                            nc.vector.tensor_copy(wo_sb[0:1, 0, 0:1],
                                                  x_t[0:1, 0:1])
                            for kk in range(G):
                                nc.gpsimd.dma_start(out=wo_sb[:, kk, :],
                                                    in_=wot_r[:, kk, :])
                    # split psum evacuation across ACT and DVE so the banks
                    # free up fast for the next j iteration; v first so the
                    # delayed transpose never waits on the staging copy
                    v_st = vstg.tile([128, 512], BF16)
                    nc.vector.tensor_copy(v_st, v_ps)
                    nc.scalar.activation(out=q_sb[:, 0, tsl], in_=q_ps[0], func=COPY)
                    nc.vector.tensor_copy(q_sb[:, 1, tsl], q_ps[1])
                    nc.scalar.activation(out=q_sb[:, 2, tsl], in_=q_ps[2], func=COPY)
                    nc.vector.tensor_copy(q_sb[:, 3, tsl], q_ps[3])
                    nc.scalar.activation(out=k_sb[:, tsl], in_=k_ps, func=COPY)
                    prev_v = (j, v_st)
                v_transpose(*prev_v)

            # ------- phase 2: fused attention + output projection -------
            with ExitStack() as c2:
                apool = c2.enter_context(tc.tile_pool(name="apool", bufs=2))
                ppool = c2.enter_context(tc.tile_pool(name="ppool", bufs=4))
                tpool = c2.enter_context(tc.tile_pool(name="tpool", bufs=2))
                rpool = c2.enter_context(tc.tile_pool(name="rpool", bufs=2))
                opool = c2.enter_context(tc.tile_pool(name="opool", bufs=3))
                psS = c2.enter_context(tc.tile_pool(name="psS", bufs=2, space="PSUM"))
                psPV = c2.enter_context(tc.tile_pool(name="psPV", bufs=2, space="PSUM"))
                psO = c2.enter_context(tc.tile_pool(name="psO", bufs=2, space="PSUM"))

                def outproj_nchunk(pb, pj, pa, tt2, n):
                    # output projection for tq-tile tt2, H-chunk n, of chunk
                    # (pb, pj): accumulate the 4 heads in PSUM against woT.
                    # Evacuations go to DVE/gpsimd so ACT stays exp-only.
                    t0 = pb * S + pj * 512 + tt2 * 128
                    o_ps = psO.tile([128, 512], F32)
                    for m in range(G):
                        nc.tensor.matmul(
                            o_ps, pa[m][:, tt2 * 128:(tt2 + 1) * 128],
                            wo_sb[:, m, n * 512:(n + 1) * 512],
                            start=(m == 0), stop=(m == G - 1))
                    o_t = opool.tile([128, 512], BF16)
                    if n % 2 == 0:
                        nc.vector.tensor_copy(o_t, o_ps)
                    else:
                        nc.scalar.activation(out=o_t, in_=o_ps, func=COPY)
                    nc.sync.dma_start(
                        out=out_r[t0:t0 + 128, n * 512:(n + 1) * 512],
                        in_=o_t)

                def flush_den(pend):
                    # softmax denominator of a finished head: one ones-matmul
                    # on the DVE tree sum, reciprocal, normalize into a_ch
                    acc512, pv_ps, a_t = pend
                    den_ps = psS.tile([128, 1024], F32, name="s_ps")
                    nc.tensor.matmul(den_ps[:, 0:512], ones_sb, acc512,
                                     start=True, stop=True)
                    rec_t = rpool.tile([128, 512], F32)
                    nc.vector.reciprocal_approx_fast(out=rec_t,
                                                     in_=den_ps[:, 0:512])
                    nc.vector.tensor_mul(a_t, pv_ps, rec_t)

                pending = None   # den work of the previous head
                prev = None      # a_ch of the previous (b, j)
                for b in range(B):
                    for j in range(SJ):
                        tqsl = slice(b * S + j * 512, b * S + (j + 1) * 512)
                        a_ch = [apool.tile([128, 512], BF16, name=f"a_ch{m}")
                                for m in range(G)]
                        for m in range(G):
                            pv_ps = psPV.tile([128, 512], F32)
                            p_list = []
                            t_parts = []
                            for g in range(NG):
                                s_ps = psS.tile([128, 1024], F32, name="s_ps")
                                for h in range(2):
                                    ti = b * SI + 2 * g + h
                                    nc.tensor.matmul(
                                        s_ps[:, h * 512:(h + 1) * 512],
                                        k_sb[:, ti * 128:(ti + 1) * 128],
                                        q_sb[:, m, tqsl], start=True, stop=True)
                                p_t = ppool.tile([128, 1024], BF16)
                                nc.scalar.activation(out=p_t, in_=s_ps,
                                                     func=EXP, scale=SCALE)
                                p_list.append(p_t)
                                if g % 2 == 1:
                                    tk = tpool.tile([128, 1024], BF16,
                                                    name=f"t{g // 2}")
                                    nc.vector.tensor_add(tk, p_list[g - 1],
                                                         p_list[g])
                                    t_parts.append(tk)
                                # den flush + interleaved output projection
                                # go BEFORE the PV pair: the PE is in-order,
                                # so exp-independent work must sit ahead of
                                # the exp-dependent PV matmuls to cover the
                                # ACT latency
                                if g == 1 and pending is not None:
                                    flush_den(pending)
                                    pending = None
                                if g >= 2 and prev is not None:
                                    outproj_nchunk(prev[0], prev[1], prev[2],
                                                   m, g - 2)
                                if g >= 1:
                                    pg = p_list[g - 1]
                                    for h in range(2):
                                        ti = b * SI + 2 * (g - 1) + h
                                        nc.tensor.matmul(
                                            pv_ps, v_sb[:, ti, :],
                                            pg[:, h * 512:(h + 1) * 512],
                                            start=(g == 1 and h == 0),
                                            stop=False)
                            if prev is not None:
                                outproj_nchunk(prev[0], prev[1], prev[2],
                                               m, NJ - 2)
                            pg = p_list[NG - 1]
                            for h in range(2):
                                ti = b * SI + 2 * (NG - 1) + h
                                nc.tensor.matmul(
                                    pv_ps, v_sb[:, ti, :],
                                    pg[:, h * 512:(h + 1) * 512],
                                    start=False, stop=(h == 1))
                            # finish the denominator tree on DVE
                            s0 = tpool.tile([128, 1024], BF16, name="s0")
                            s1 = tpool.tile([128, 1024], BF16, name="s1")
                            nc.vector.tensor_add(s0, t_parts[0], t_parts[1])
                            nc.vector.tensor_add(s1, t_parts[2], t_parts[3])
                            a1024 = tpool.tile([128, 1024], BF16, name="a1024")
                            nc.vector.tensor_add(a1024, s0, s1)
                            acc512 = tpool.tile([128, 512], BF16, name="a512")
                            nc.vector.tensor_add(acc512, a1024[:, 0:512],
                                                 a1024[:, 512:1024])
                            pending = (acc512, pv_ps, a_ch[m])
                            if prev is not None:
                                outproj_nchunk(prev[0], prev[1], prev[2],
                                               m, NJ - 1)
                        prev = (b, j, a_ch)
                flush_den(pending)
                for tt2 in range(4):
                    for n in range(NJ):
                        outproj_nchunk(prev[0], prev[1], prev[2], tt2, n)
    nc.compile()
    return nc


_NC_CACHE = None


def _get_nc():
    global _NC_CACHE
    if _NC_CACHE is None:
        _NC_CACHE = build_nc()
    return _NC_CACHE


def make_in_maps(x, wq, wk, wv, wo):
    xt = np.ascontiguousarray(x.reshape(T, H).T).astype(bfloat16)
    wqb = wq.astype(bfloat16)
    wkb = wk.astype(bfloat16)
    wvb = wv.astype(bfloat16)
    wob = wo.astype(bfloat16)
    ones = np.ones((128, 128), dtype=bfloat16)
    in_maps = []
    for c in range(NCORES):
        qsl = slice(c * G * D, (c + 1) * G * D)
        ksl = slice(c * D, (c + 1) * D)
        in_maps.append({
            "xt": xt,
            "wqt": np.ascontiguousarray(wqb[qsl, :].T),
            "wkt": np.ascontiguousarray(wkb[ksl, :].T),
            "wvt": np.ascontiguousarray(wvb[ksl, :].T),
            "wot": np.ascontiguousarray(wob[:, qsl].T),
            "ones": ones,
        })
    return in_maps


def kernel(x, wq, wk, wv, wo, **run_kwargs):
    nc = _get_nc()
    in_maps = make_in_maps(np.asarray(x, dtype=np.float32),
                           np.asarray(wq, dtype=np.float32),
                           np.asarray(wk, dtype=np.float32),
                           np.asarray(wv, dtype=np.float32),
                           np.asarray(wo, dtype=np.float32))
    res = run_bass_kernel_spmd(nc, in_maps, core_ids=list(range(NCORES)),
                               **run_kwargs)
    acc = np.zeros((T, H), dtype=np.float32)
    for c in range(NCORES):
        acc += res.results[c]["out"].astype(np.float32)
    out = acc.reshape(B, S, H)
    if run_kwargs:
        return out, res
    return out


# revision 22
# speedup vs baseline: 1.1450x; 1.1450x over previous
"""Trainium2 Bass kernel for Llama GQA attention (no mask), 8-way tensor
parallel over KV heads.

Problem shapes (hardcoded):
  x  (2, 2048, 4096) f32
  wq (4096, 4096), wk (1024, 4096), wv (1024, 4096), wo (4096, 4096) f32
  NUM_HEADS=32, NUM_KV_HEADS=8, HEAD_DIM=128, GQA group g=4

Sharding: core c owns KV head c (4 Q heads). x replicated (pre-transposed
to xT on host), wq/wk/wv sharded on output dim (pre-transposed host-side),
wo sharded on input dim. Each core computes a partial (4096, 4096) output
(its heads' contribution through wo); host sums the 8 partials in fp32.

All tensors are bf16 (PSUM accumulation fp32): same PE rate as fp32r
(1 col/cycle) but half the DMA/SBUF traffic, which removes the phase-1
x-feed stalls the fp32 version had.

Structure:
  phase 1: q/k/v projections. Weights DMA'd per-k-tile on the gpsimd
    queue, wo prefetched right after; x tiles on the sync queue.
    vT -> v via PE transposes. PSUM evacuations split ACT/DVE.
  phase 2 (fused attention + output projection, software-pipelined):
    per (batch, tq-chunk, head): scores transposed ST = kT_tile.T @ qT
    into [128,1024] PSUM (2 k-tiles per matmul pair), batched exp ->
    p (bf16). Softmax denominator = DVE pairwise-tree sum of the 16 p
    tiles + ONE ones-matmul per head (instead of 16 PE den matmuls).
    S-pairs are emitted one step ahead of PV-pairs so the PE never
    waits on the ACT exp; den/rec/normalize of head m flush during
    head m+1; output-projection quarters of the previous (b,chunk)
    interleave after each head so DVE/ACT load stays smooth.
"""

import sys
from contextlib import ExitStack

import numpy as np
from ml_dtypes import bfloat16

sys.path.insert(0, "/opt/trn_rl_repo")

import concourse.bass as bass  # noqa: E402
import concourse.tile as tile  # noqa: E402
from concourse import bacc, mybir  # noqa: E402
from concourse.bass_utils import run_bass_kernel_spmd  # noqa: E402
from concourse.masks import make_identity  # noqa: E402

NCORES = 8
B, S, H = 2, 2048, 4096
T = B * S                      # 4096 flattened tokens
D = 128                        # head dim
G = 4                          # q heads per core (GQA group)
HK = 32                        # h k-tiles (4096 / 128)
TT = T // 128                  # 32 token tiles
NJ = T // 512                  # 8 token chunks of 512
SJ = S // 512                  # 4 tq chunks per batch
SI = S // 128                  # 16 tk tiles per batch
NG = SI // 2                   # 8 k-tile pairs per batch
SCALE = float(1.0 / np.sqrt(D))

F32 = mybir.dt.float32
BF16 = mybir.dt.bfloat16
COPY = mybir.ActivationFunctionType.Copy
EXP = mybir.ActivationFunctionType.Exp


def build_nc():
    nc = bacc.Bacc("TRN2", target_bir_lowering=False, debug=False,
                   enable_asserts=True, num_devices=NCORES)
    xt = nc.declare_dram_parameter("xt", [H, T], BF16, isOutput=False)
    wqt = nc.declare_dram_parameter("wqt", [H, G * D], BF16, isOutput=False)
    wkt = nc.declare_dram_parameter("wkt", [H, D], BF16, isOutput=False)
    wvt = nc.declare_dram_parameter("wvt", [H, D], BF16, isOutput=False)
    wot = nc.declare_dram_parameter("wot", [G * D, H], BF16, isOutput=False)
    ones = nc.declare_dram_parameter("ones", [128, 128], BF16, isOutput=False)
    out = nc.declare_dram_parameter("out", [T, H], BF16, isOutput=True)

    xt_r = xt.ap().rearrange("(k p) t -> p k t", p=128)     # [128, 32, T]
    wqt_r = wqt.ap().rearrange("(k p) m -> p k m", p=128)   # [128, 32, 512]
    wkt_r = wkt.ap().rearrange("(k p) m -> p k m", p=128)   # [128, 32, 128]
    wvt_r = wvt.ap().rearrange("(k p) m -> p k m", p=128)   # [128, 32, 128]
    wot_r = wot.ap().rearrange("(k p) n -> p k n", p=128)   # [128, 4, T]
    out_r = out.ap()

    with tile.TileContext(nc) as tc:
        with ExitStack() as ctx:
            persist = ctx.enter_context(tc.tile_pool(name="persist", bufs=1))
            q_sb = persist.tile([128, G, T], BF16)       # qT per head, 4MB
            k_sb = persist.tile([128, T], BF16)          # kT, 1MB
            v_sb = persist.tile([128, TT, D], BF16)      # v natural, 1MB
            wo_sb = persist.tile([128, G, T], BF16)      # woT resident, 4MB
            ones_sb = persist.tile([128, 128], BF16)
            nc.scalar.dma_start(out=ones_sb, in_=ones.ap())

            # ---------------- phase 1: projections ----------------
            with ExitStack() as c1:
                wpool = c1.enter_context(tc.tile_pool(name="wpool", bufs=1))
                xpool = c1.enter_context(tc.tile_pool(name="xpool", bufs=12))
                vstg = c1.enter_context(tc.tile_pool(name="vstg", bufs=2))
                ps1 = c1.enter_context(tc.tile_pool(name="ps1", bufs=1, space="PSUM"))
                pstr = c1.enter_context(tc.tile_pool(name="pstr", bufs=2, space="PSUM"))

                # one tile PER k-chunk: dependency tracking is
                # tile-granular, so a single big tile would make the first
                # matmul wait for ALL 32 chunk DMAs
                wq_t = [wpool.tile([128, G * D], BF16, name=f"wq{k}")
                        for k in range(HK)]
                wk_t = [wpool.tile([128, D], BF16, name=f"wk{k}")
                        for k in range(HK)]
                wv_t = [wpool.tile([128, D], BF16, name=f"wv{k}")
                        for k in range(HK)]
                ident = wpool.tile([128, 128], BF16)
                dummy = wpool.tile([1, 1], BF16)
                for k in range(HK):
                    wq_q = nc.gpsimd if k % 2 == 0 else nc.scalar
                    wq_q.dma_start(out=wq_t[k], in_=wqt_r[:, k, :])
                    nc.gpsimd.dma_start(out=wk_t[k], in_=wkt_r[:, k, :])
                    nc.scalar.dma_start(out=wv_t[k], in_=wvt_r[:, k, :])
                make_identity(nc, ident)

                def v_transpose(pj, pv_st):
                    # one-j-delayed so PE never waits on the DVE staging copy
                    vt_ps = pstr.tile([128, 4, 128], BF16)
                    for tt in range(4):
                        nc.tensor.transpose(
                            vt_ps[:, tt, :], pv_st[:, tt * 128:(tt + 1) * 128],
                            ident)
                    nc.scalar.activation(
                        out=v_sb[:, 4 * pj:4 * pj + 4, :], in_=vt_ps, func=COPY)

                prev_v = None
                for j in range(NJ):
                    tsl = slice(j * 512, (j + 1) * 512)
                    q_ps = [ps1.tile([128, 512], F32, name=f"q_ps{m}")
                            for m in range(G)]
                    k_ps = ps1.tile([128, 512], F32)
                    v_ps = ps1.tile([128, 512], F32)
                    for k in range(HK):
                        x_t = xpool.tile([128, 512], BF16)
                        nc.sync.dma_start(out=x_t, in_=xt_r[:, k, tsl])
                        st = k == 0
                        sp = k == HK - 1
                        for m in range(G):
                            nc.tensor.matmul(
                                q_ps[m], wq_t[k][:, m * D:(m + 1) * D], x_t,
                                start=st, stop=sp)
                        nc.tensor.matmul(k_ps, wk_t[k], x_t, start=st, stop=sp)
                        nc.tensor.matmul(v_ps, wv_t[k], x_t, start=st, stop=sp)
                        if k == 2 and prev_v is not None:
                            v_transpose(*prev_v)
                        # prefetch wo for phase 2, gated on a j==3 x tile.
                        # The gpsimd queue posts DMAs in relaxed order, so a
                        # copy BEFORE the dma_start does not delay it; a
                        # writer-after-writer dependency on wo_sb itself does.
                        if j == 3 and k == 0:
                            nc.vector.tensor_copy(wo_sb[0:1, 0, 0:1],
                                                  x_t[0:1, 0:1])
                            for kk in range(G):
                                nc.gpsimd.dma_start(out=wo_sb[:, kk, :],
                                                    in_=wot_r[:, kk, :])
                    # split psum evacuation across ACT and DVE so the banks
                    # free up fast for the next j iteration; v first so the
                    # delayed transpose never waits on the staging copy
                    v_st = vstg.tile([128, 512], BF16)
                    nc.vector.tensor_copy(v_st, v_ps)
                    nc.scalar.activation(out=q_sb[:, 0, tsl], in_=q_ps[0], func=COPY)
                    nc.vector.tensor_copy(q_sb[:, 1, tsl], q_ps[1])
                    nc.scalar.activation(out=q_sb[:, 2, tsl], in_=q_ps[2], func=COPY)
                    nc.vector.tensor_copy(q_sb[:, 3, tsl], q_ps[3])
                    nc.scalar.activation(out=k_sb[:, tsl], in_=k_ps, func=COPY)
                    prev_v = (j, v_st)
                v_transpose(*prev_v)

            # ------- phase 2: fused attention + output projection -------
            with ExitStack() as c2:
                apool = c2.enter_context(tc.tile_pool(name="apool", bufs=2))
                ppool = c2.enter_context(tc.tile_pool(name="ppool", bufs=4))
                tpool = c2.enter_context(tc.tile_pool(name="tpool", bufs=2))
                rpool = c2.enter_context(tc.tile_pool(name="rpool", bufs=2))
                opool = c2.enter_context(tc.tile_pool(name="opool", bufs=3))
                psS = c2.enter_context(tc.tile_pool(name="psS", bufs=2, space="PSUM"))
                psPV = c2.enter_context(tc.tile_pool(name="psPV", bufs=2, space="PSUM"))
                psO = c2.enter_context(tc.tile_pool(name="psO", bufs=2, space="PSUM"))

                def outproj_nchunk(pb, pj, pa, tt2, n):
                    # output projection for tq-tile tt2, H-chunk n, of chunk
                    # (pb, pj): accumulate the 4 heads in PSUM against woT.
                    # Evacuations go to DVE/gpsimd so ACT stays exp-only.
                    t0 = pb * S + pj * 512 + tt2 * 128
                    o_ps = psO.tile([128, 512], F32)
                    for m in range(G):
                        nc.tensor.matmul(
                            o_ps, pa[m][:, tt2 * 128:(tt2 + 1) * 128],
                            wo_sb[:, m, n * 512:(n + 1) * 512],
                            start=(m == 0), stop=(m == G - 1))
                    o_t = opool.tile([128, 512], BF16)
                    if n % 2 == 0:
                        nc.vector.tensor_copy(o_t, o_ps)
                    else:
                        nc.scalar.activation(out=o_t, in_=o_ps, func=COPY)
                    nc.sync.dma_start(
                        out=out_r[t0:t0 + 128, n * 512:(n + 1) * 512],
                        in_=o_t)

                def flush_den(pend):
                    # softmax denominator of a finished head: one ones-matmul
                    # on the DVE tree sum, reciprocal, normalize into a_ch
                    acc512, pv_ps, a_t = pend
                    den_ps = psS.tile([128, 1024], F32, name="s_ps")
                    nc.tensor.matmul(den_ps[:, 0:512], ones_sb, acc512,
                                     start=True, stop=True)
                    rec_t = rpool.tile([128, 512], F32)
                    nc.vector.reciprocal_approx_fast(out=rec_t,
                                                     in_=den_ps[:, 0:512])
                    nc.vector.tensor_mul(a_t, pv_ps, rec_t)

                pending = None   # den work of the previous head
                prev = None      # a_ch of the previous (b, j)
                for b in range(B):
                    for j in range(SJ):
                        tqsl = slice(b * S + j * 512, b * S + (j + 1) * 512)
                        a_ch = [apool.tile([128, 512], BF16, name=f"a_ch{m}")
                                for m in range(G)]
                        for m in range(G):
                            pv_ps = psPV.tile([128, 512], F32)
                            p_list = []
                            t_parts = []
                            for g in range(NG):
                                s_ps = psS.tile([128, 1024], F32, name="s_ps")
                                for h in range(2):
                                    ti = b * SI + 2 * g + h
                                    nc.tensor.matmul(
                                        s_ps[:, h * 512:(h + 1) * 512],
                                        k_sb[:, ti * 128:(ti + 1) * 128],
                                        q_sb[:, m, tqsl], start=True, stop=True)
                                p_t = ppool.tile([128, 1024], BF16)
                                nc.scalar.activation(out=p_t, in_=s_ps,
                                                     func=EXP, scale=SCALE)
                                p_list.append(p_t)
                                if g % 2 == 1:
                                    tk = tpool.tile([128, 1024], BF16,
                                                    name=f"t{g // 2}")
                                    nc.vector.tensor_add(tk, p_list[g - 1],
                                                         p_list[g])
                                    t_parts.append(tk)
                                # den flush + interleaved output projection
                                # go BEFORE the PV pair: the PE is in-order,
                                # so exp-independent work must sit ahead of
                                # the exp-dependent PV matmuls to cover the
                                # ACT latency
                                if g == 1 and pending is not None:
                                    flush_den(pending)
                                    pending = None
                                if g >= 2 and prev is not None:
                                    outproj_nchunk(prev[0], prev[1], prev[2],
                                                   m, g - 2)
                                if g >= 1:
                                    pg = p_list[g - 1]
                                    for h in range(2):
                                        ti = b * SI + 2 * (g - 1) + h
                                        nc.tensor.matmul(
                                            pv_ps, v_sb[:, ti, :],
                                            pg[:, h * 512:(h + 1) * 512],
                                            start=(g == 1 and h == 0),
                                            stop=False)
                            if prev is not None:
                                outproj_nchunk(prev[0], prev[1], prev[2],
                                               m, NJ - 2)
                            pg = p_list[NG - 1]
                            for h in range(2):
                                ti = b * SI + 2 * (NG - 1) + h
                                nc.tensor.matmul(
                                    pv_ps, v_sb[:, ti, :],
                                    pg[:, h * 512:(h + 1) * 512],
                                    start=False, stop=(h == 1))
                            # finish the denominator tree on DVE
                            s0 = tpool.tile([128, 1024], BF16, name="s0")
                            s1 = tpool.tile([128, 1024], BF16, name="s1")
                            nc.vector.tensor_add(s0, t_parts[0], t_parts[1])
                            nc.vector.tensor_add(s1, t_parts[2], t_parts[3])
                            a1024 = tpool.tile([128, 1024], BF16, name="a1024")
                            nc.vector.tensor_add(a1024, s0, s1)
                            acc512 = tpool.tile([128, 512], BF16, name="a512")
                            nc.vector.tensor_add(acc512, a1024[:, 0:512],
                                                 a1024[:, 512:1024])
                            pending = (acc512, pv_ps, a_ch[m])
                            if prev is not None:
                                outproj_nchunk(prev[0], prev[1], prev[2],
                                               m, NJ - 1)
                        prev = (b, j, a_ch)
                flush_den(pending)
                for tt2 in range(4):
                    for n in range(NJ):
                        outproj_nchunk(prev[0], prev[1], prev[2], tt2, n)
    nc.compile()
    return nc


_NC_CACHE = None


def _get_nc():
    global _NC_CACHE
    if _NC_CACHE is None:
        _NC_CACHE = build_nc()
    return _NC_CACHE


def make_in_maps(x, wq, wk, wv, wo):
    xt = np.ascontiguousarray(x.reshape(T, H).T).astype(bfloat16)
    wqb = wq.astype(bfloat16)
    wkb = wk.astype(bfloat16)
    wvb = wv.astype(bfloat16)
    wob = wo.astype(bfloat16)
    ones = np.ones((128, 128), dtype=bfloat16)
    in_maps = []
    for c in range(NCORES):
        qsl = slice(c * G * D, (c + 1) * G * D)
        ksl = slice(c * D, (c + 1) * D)
        in_maps.append({
            "xt": xt,
            "wqt": np.ascontiguousarray(wqb[qsl, :].T),
            "wkt": np.ascontiguousarray(wkb[ksl, :].T),
            "wvt": np.ascontiguousarray(wvb[ksl, :].T),
            "wot": np.ascontiguousarray(wob[:, qsl].T),
            "ones": ones,
        })
    return in_maps


def kernel(x, wq, wk, wv, wo, **run_kwargs):
    nc = _get_nc()
    in_maps = make_in_maps(np.asarray(x, dtype=np.float32),
                           np.asarray(wq, dtype=np.float32),
                           np.asarray(wk, dtype=np.float32),
                           np.asarray(wv, dtype=np.float32),
                           np.asarray(wo, dtype=np.float32))
    res = run_bass_kernel_spmd(nc, in_maps, core_ids=list(range(NCORES)),
                               **run_kwargs)
    acc = np.zeros((T, H), dtype=np.float32)
    for c in range(NCORES):
        acc += res.results[c]["out"].astype(np.float32)
    out = acc.reshape(B, S, H)
    if run_kwargs:
        return out, res
    return out


# revision 30
# speedup vs baseline: 1.1862x; 1.0360x over previous
"""Trainium2 Bass kernel for Llama GQA attention (no mask), 8-way tensor
parallel over KV heads.

Problem shapes (hardcoded):
  x  (2, 2048, 4096) f32
  wq (4096, 4096), wk (1024, 4096), wv (1024, 4096), wo (4096, 4096) f32
  NUM_HEADS=32, NUM_KV_HEADS=8, HEAD_DIM=128, GQA group g=4

Sharding: core c owns KV head c (4 Q heads). x replicated (pre-transposed
to xT on host), wq/wk/wv sharded on output dim (pre-transposed host-side),
wo sharded on input dim. Each core computes a partial (4096, 4096) output
(its heads' contribution through wo); host sums the 8 partials in fp32.

All tensors are bf16 (PSUM accumulation fp32): same PE rate as fp32r
(1 col/cycle) but half the DMA/SBUF traffic, which removes the phase-1
x-feed stalls the fp32 version had.

Structure:
  phase 1: q/k/v projections. Weight chunk k (wq+wk+wv) DMA'd as
    per-k tiles (dependency tracking is tile-granular) alternating
    gpsimd/scalar queues; x tiles on the sync queue (12-deep ring).
    wo prefetch is WAR-gated on a j==3 x tile so the run-ahead DMA
    queues can't flood the startup window. vT -> v via PE transposes.
    PSUM evacuations split ACT/DVE, k first (phase-2 boundary).
  phase 2 (fused attention + output projection, software-pipelined):
    per (batch, tq-chunk, head): scores transposed ST = kT_tile.T @ qT
    into [128,1024] PSUM (2 k-tiles per matmul pair), batched exp ->
    p (bf16). Softmax denominator = DVE pairwise-tree sum of the 16 p
    tiles + ONE ones-matmul per head (instead of 16 PE den matmuls).
    The PE is in-order, so per g the emission is: S-pair(g), then
    exp-independent filler (den flush of the previous head at g==1 /
    one output-projection H-chunk of the previous (b,chunk)), then
    PV-pair(g-1) — the filler covers the ACT exp latency. The last
    (b,chunk)'s output projection drains in a tail block.
"""

import sys
from contextlib import ExitStack

import numpy as np
from ml_dtypes import bfloat16

sys.path.insert(0, "/opt/trn_rl_repo")

import concourse.bass as bass  # noqa: E402
import concourse.tile as tile  # noqa: E402
from concourse import bacc, mybir  # noqa: E402
from concourse.bass_utils import run_bass_kernel_spmd  # noqa: E402
from concourse.masks import make_identity  # noqa: E402

NCORES = 8
B, S, H = 2, 2048, 4096
T = B * S                      # 4096 flattened tokens
D = 128                        # head dim
G = 4                          # q heads per core (GQA group)
HK = 32                        # h k-tiles (4096 / 128)
TT = T // 128                  # 32 token tiles
NJ = T // 512                  # 8 token chunks of 512
SJ = S // 512                  # 4 tq chunks per batch
SI = S // 128                  # 16 tk tiles per batch
NG = SI // 2                   # 8 k-tile pairs per batch
SCALE = float(1.0 / np.sqrt(D))

F32 = mybir.dt.float32
BF16 = mybir.dt.bfloat16
COPY = mybir.ActivationFunctionType.Copy
EXP = mybir.ActivationFunctionType.Exp


def build_nc():
    nc = bacc.Bacc("TRN2", target_bir_lowering=False, debug=False,
                   enable_asserts=True, num_devices=NCORES)
    xt = nc.declare_dram_parameter("xt", [H, T], BF16, isOutput=False)
    wqt = nc.declare_dram_parameter("wqt", [H, G * D], BF16, isOutput=False)
    wkt = nc.declare_dram_parameter("wkt", [H, D], BF16, isOutput=False)
    wvt = nc.declare_dram_parameter("wvt", [H, D], BF16, isOutput=False)
    wot = nc.declare_dram_parameter("wot", [G * D, H], BF16, isOutput=False)
    ones = nc.declare_dram_parameter("ones", [128, 128], BF16, isOutput=False)
    out = nc.declare_dram_parameter("out", [T, H], BF16, isOutput=True)

    xt_r = xt.ap().rearrange("(k p) t -> p k t", p=128)     # [128, 32, T]
    wqt_r = wqt.ap().rearrange("(k p) m -> p k m", p=128)   # [128, 32, 512]
    wkt_r = wkt.ap().rearrange("(k p) m -> p k m", p=128)   # [128, 32, 128]
    wvt_r = wvt.ap().rearrange("(k p) m -> p k m", p=128)   # [128, 32, 128]
    wot_r = wot.ap().rearrange("(k p) n -> p k n", p=128)   # [128, 4, T]
    out_r = out.ap()

    with tile.TileContext(nc) as tc:
        with ExitStack() as ctx:
            persist = ctx.enter_context(tc.tile_pool(name="persist", bufs=1))
            q_sb = persist.tile([128, G, T], BF16)       # qT per head, 4MB
            k_sb = persist.tile([128, T], BF16)          # kT, 1MB
            v_sb = persist.tile([128, TT, D], BF16)      # v natural, 1MB
            wo_sb = persist.tile([128, G, T], BF16)      # woT resident, 4MB
            ones_sb = persist.tile([128, 128], BF16)
            nc.scalar.dma_start(out=ones_sb, in_=ones.ap())

            # ---------------- phase 1: projections ----------------
            with ExitStack() as c1:
                wpool = c1.enter_context(tc.tile_pool(name="wpool", bufs=1))
                xpool = c1.enter_context(tc.tile_pool(name="xpool", bufs=12))
                vstg = c1.enter_context(tc.tile_pool(name="vstg", bufs=2))
                ps1 = c1.enter_context(tc.tile_pool(name="ps1", bufs=1, space="PSUM"))
                pstr = c1.enter_context(tc.tile_pool(name="pstr", bufs=2, space="PSUM"))

                # one tile PER k-chunk: dependency tracking is
                # tile-granular, so a single big tile would make the first
                # matmul wait for ALL 32 chunk DMAs
                wq_t = [wpool.tile([128, G * D], BF16, name=f"wq{k}")
                        for k in range(HK)]
                wk_t = [wpool.tile([128, D], BF16, name=f"wk{k}")
                        for k in range(HK)]
                wv_t = [wpool.tile([128, D], BF16, name=f"wv{k}")
                        for k in range(HK)]
                ident = wpool.tile([128, 128], BF16)
                # all three tensors of chunk k go to the SAME queue,
                # alternating by k, so complete chunks arrive in the order
                # the j=0 matmuls consume them
                for k in range(HK):
                    q = nc.gpsimd if k % 2 == 0 else nc.scalar
                    q.dma_start(out=wq_t[k], in_=wqt_r[:, k, :])
                    q.dma_start(out=wk_t[k], in_=wkt_r[:, k, :])
                    q.dma_start(out=wv_t[k], in_=wvt_r[:, k, :])
                make_identity(nc, ident)

                def v_transpose(pj, pv_st):
                    # one-j-delayed so PE never waits on the DVE staging copy
                    vt_ps = pstr.tile([128, 4, 128], BF16)
                    for tt in range(4):
                        nc.tensor.transpose(
                            vt_ps[:, tt, :], pv_st[:, tt * 128:(tt + 1) * 128],
                            ident)
                    nc.scalar.activation(
                        out=v_sb[:, 4 * pj:4 * pj + 4, :], in_=vt_ps, func=COPY)

                prev_v = None
                for j in range(NJ):
                    tsl = slice(j * 512, (j + 1) * 512)
                    q_ps = [ps1.tile([128, 512], F32, name=f"q_ps{m}")
                            for m in range(G)]
                    k_ps = ps1.tile([128, 512], F32)
                    v_ps = ps1.tile([128, 512], F32)
                    for k in range(HK):
                        x_t = xpool.tile([128, 512], BF16)
                        nc.sync.dma_start(out=x_t, in_=xt_r[:, k, tsl])
                        st = k == 0
                        sp = k == HK - 1
                        for m in range(G):
                            nc.tensor.matmul(
                                q_ps[m], wq_t[k][:, m * D:(m + 1) * D], x_t,
                                start=st, stop=sp)
                        nc.tensor.matmul(k_ps, wk_t[k], x_t, start=st, stop=sp)
                        nc.tensor.matmul(v_ps, wv_t[k], x_t, start=st, stop=sp)
                        if k == 2 and prev_v is not None:
                            v_transpose(*prev_v)
                        # prefetch wo for phase 2, gated on a j==3 x tile.
                        # The gpsimd queue posts DMAs in relaxed order, so a
                        # copy BEFORE the dma_start does not delay it; a
                        # writer-after-writer dependency on wo_sb itself does.
                        if j == 3 and k == 0:
                            nc.vector.tensor_copy(wo_sb[0:1, 0, 0:1],
                                                  x_t[0:1, 0:1])
                            for kk in range(G):
                                nc.gpsimd.dma_start(out=wo_sb[:, kk, :],
                                                    in_=wot_r[:, kk, :])
                    # split psum evacuation across ACT and DVE so the banks
                    # free up fast for the next j iteration; v first so the
                    # delayed transpose never waits on the staging copy
                    v_st = vstg.tile([128, 512], BF16)
                    nc.vector.tensor_copy(v_st, v_ps)
                    nc.scalar.activation(out=k_sb[:, tsl], in_=k_ps, func=COPY)
                    nc.scalar.activation(out=q_sb[:, 0, tsl], in_=q_ps[0], func=COPY)
                    nc.vector.tensor_copy(q_sb[:, 1, tsl], q_ps[1])
                    nc.scalar.activation(out=q_sb[:, 2, tsl], in_=q_ps[2], func=COPY)
                    nc.vector.tensor_copy(q_sb[:, 3, tsl], q_ps[3])
                    prev_v = (j, v_st)
                v_transpose(*prev_v)

            # ------- phase 2: fused attention + output projection -------
            with ExitStack() as c2:
                apool = c2.enter_context(tc.tile_pool(name="apool", bufs=2))
                ppool = c2.enter_context(tc.tile_pool(name="ppool", bufs=4))
                tpool = c2.enter_context(tc.tile_pool(name="tpool", bufs=2))
                rpool = c2.enter_context(tc.tile_pool(name="rpool", bufs=2))
                opool = c2.enter_context(tc.tile_pool(name="opool", bufs=3))
                psS = c2.enter_context(tc.tile_pool(name="psS", bufs=2, space="PSUM"))
                psPV = c2.enter_context(tc.tile_pool(name="psPV", bufs=2, space="PSUM"))
                psO = c2.enter_context(tc.tile_pool(name="psO", bufs=2, space="PSUM"))

                def outproj_nchunk(pb, pj, pa, tt2, n, tail=False):
                    # output projection for tq-tile tt2, H-chunk n, of chunk
                    # (pb, pj): accumulate the 4 heads in PSUM against woT.
                    # Evacuations go to DVE/gpsimd so ACT stays exp-only.
                    t0 = pb * S + pj * 512 + tt2 * 128
                    o_ps = psO.tile([128, 512], F32)
                    for m in range(G):
                        nc.tensor.matmul(
                            o_ps, pa[m][:, tt2 * 128:(tt2 + 1) * 128],
                            wo_sb[:, m, n * 512:(n + 1) * 512],
                            start=(m == 0), stop=(m == G - 1))
                    o_t = opool.tile([128, 512], BF16)
                    if n % 2 == 0:
                        nc.vector.tensor_copy(o_t, o_ps)
                    else:
                        nc.scalar.activation(out=o_t, in_=o_ps, func=COPY)
                    nc.sync.dma_start(
                        out=out_r[t0:t0 + 128, n * 512:(n + 1) * 512],
                        in_=o_t)

                def flush_den(pend):
                    # softmax denominator of a finished head: one ones-matmul
                    # on the DVE tree sum, reciprocal, normalize into a_ch
                    acc512, pv_ps, a_t = pend
                    den_ps = psS.tile([128, 1024], F32, name="s_ps")
                    nc.tensor.matmul(den_ps[:, 0:512], ones_sb, acc512,
                                     start=True, stop=True)
                    rec_t = rpool.tile([128, 512], F32)
                    nc.vector.reciprocal_approx_fast(out=rec_t,
                                                     in_=den_ps[:, 0:512])
                    nc.vector.tensor_mul(a_t, pv_ps, rec_t)

                pending = None   # den work of the previous head
                prev = None      # a_ch of the previous (b, j)
                for b in range(B):
                    for j in range(SJ):
                        tqsl = slice(b * S + j * 512, b * S + (j + 1) * 512)
                        a_ch = [apool.tile([128, 512], BF16, name=f"a_ch{m}")
                                for m in range(G)]
                        for m in range(G):
                            pv_ps = psPV.tile([128, 512], F32)
                            p_list = []
                            t_parts = []
                            for g in range(NG):
                                s_ps = psS.tile([128, 1024], F32, name="s_ps")
                                for h in range(2):
                                    ti = b * SI + 2 * g + h
                                    nc.tensor.matmul(
                                        s_ps[:, h * 512:(h + 1) * 512],
                                        k_sb[:, ti * 128:(ti + 1) * 128],
                                        q_sb[:, m, tqsl], start=True, stop=True)
                                p_t = ppool.tile([128, 1024], BF16)
                                nc.scalar.activation(out=p_t, in_=s_ps,
                                                     func=EXP, scale=SCALE)
                                p_list.append(p_t)
                                if g % 2 == 1:
                                    tk = tpool.tile([128, 1024], BF16,
                                                    name=f"t{g // 2}")
                                    nc.vector.tensor_add(tk, p_list[g - 1],
                                                         p_list[g])
                                    t_parts.append(tk)
                                # den flush + interleaved output projection
                                # go BEFORE the PV pair: the PE is in-order,
                                # so exp-independent work must sit ahead of
                                # the exp-dependent PV matmuls to cover the
                                # ACT latency
                                if g == 1 and pending is not None:
                                    flush_den(pending)
                                    pending = None
                                if g >= 2 and prev is not None:
                                    outproj_nchunk(prev[0], prev[1], prev[2],
                                                   m, g - 2)
                                if g >= 1:
                                    pg = p_list[g - 1]
                                    for h in range(2):
                                        ti = b * SI + 2 * (g - 1) + h
                                        nc.tensor.matmul(
                                            pv_ps, v_sb[:, ti, :],
                                            pg[:, h * 512:(h + 1) * 512],
                                            start=(g == 1 and h == 0),
                                            stop=False)
                            if prev is not None:
                                outproj_nchunk(prev[0], prev[1], prev[2],
                                               m, NJ - 2)
                            pg = p_list[NG - 1]
                            for h in range(2):
                                ti = b * SI + 2 * (NG - 1) + h
                                nc.tensor.matmul(
                                    pv_ps, v_sb[:, ti, :],
                                    pg[:, h * 512:(h + 1) * 512],
                                    start=False, stop=(h == 1))
                            # finish the denominator tree on DVE
                            s0 = tpool.tile([128, 1024], BF16, name="s0")
                            s1 = tpool.tile([128, 1024], BF16, name="s1")
                            nc.vector.tensor_add(s0, t_parts[0], t_parts[1])
                            nc.vector.tensor_add(s1, t_parts[2], t_parts[3])
                            a1024 = tpool.tile([128, 1024], BF16, name="a1024")
                            nc.vector.tensor_add(a1024, s0, s1)
                            acc512 = tpool.tile([128, 512], BF16, name="a512")
                            nc.vector.tensor_add(acc512, a1024[:, 0:512],
                                                 a1024[:, 512:1024])
                            pending = (acc512, pv_ps, a_ch[m])
                            if prev is not None:
                                outproj_nchunk(prev[0], prev[1], prev[2],
                                               m, NJ - 1)
                        prev = (b, j, a_ch)
                flush_den(pending)
                for tt2 in range(4):
                    for n in range(NJ):
                        outproj_nchunk(prev[0], prev[1], prev[2], tt2, n,
                                       tail=True)
    nc.compile()
    return nc


_NC_CACHE = None


def _get_nc():
    global _NC_CACHE
    if _NC_CACHE is None:
        _NC_CACHE = build_nc()
    return _NC_CACHE


def make_in_maps(x, wq, wk, wv, wo):
    xt = np.ascontiguousarray(x.reshape(T, H).T).astype(bfloat16)
    wqb = wq.astype(bfloat16)
    wkb = wk.astype(bfloat16)
    wvb = wv.astype(bfloat16)
    wob = wo.astype(bfloat16)
    ones = np.ones((128, 128), dtype=bfloat16)
    in_maps = []
    for c in range(NCORES):
        qsl = slice(c * G * D, (c + 1) * G * D)
        ksl = slice(c * D, (c + 1) * D)
        in_maps.append({
            "xt": xt,
            "wqt": np.ascontiguousarray(wqb[qsl, :].T),
            "wkt": np.ascontiguousarray(wkb[ksl, :].T),
            "wvt": np.ascontiguousarray(wvb[ksl, :].T),
            "wot": np.ascontiguousarray(wob[:, qsl].T),
            "ones": ones,
        })
    return in_maps


def kernel(x, wq, wk, wv, wo, **run_kwargs):
    nc = _get_nc()
    in_maps = make_in_maps(np.asarray(x, dtype=np.float32),
                           np.asarray(wq, dtype=np.float32),
                           np.asarray(wk, dtype=np.float32),
                           np.asarray(wv, dtype=np.float32),
                           np.asarray(wo, dtype=np.float32))
    res = run_bass_kernel_spmd(nc, in_maps, core_ids=list(range(NCORES)),
                               **run_kwargs)
    acc = np.zeros((T, H), dtype=np.float32)
    for c in range(NCORES):
        acc += res.results[c]["out"].astype(np.float32)
    out = acc.reshape(B, S, H)
    if run_kwargs:
        return out, res
    return out
